# revision 5
# baseline (speedup 1.0000x reference)
"""Trainium2 Bass kernel: 7x7 valid cross-correlation + bias on a 4096x4096 f32 image.

Formulation: banded matmul on the TensorEngine.
  out[r, c] = sum_{di,dj} w[di,dj] * x[r+di, c+dj]
For an output row-strip of M=122 rows starting at r0, using K=128 input rows:
  out[r0+m, c] = sum_k A_dj[k, m] * x[r0+k, c+dj]   summed over dj=0..6
where A_dj[k, m] = w[k-m, dj] for 0 <= k-m < 7 (a banded [128, 122] matrix,
precomputed on host from the 49 kernel weights). The 7 dj-terms accumulate
into one PSUM bank via shifted column slices of the same SBUF rhs tile.

Matmuls run in bf16; output written back as bf16, upcast on host
(rel-err ~4e-3 vs the 2e-2 gate).

Schedule (v4, trace-driven):
  - exec_time spans first USER instruction -> end of NEFF epilogue; the
    framework preamble is free, the trailing semaphore sweep is not.
  - HAM warmup: dummy matmuls on a memset scratch during the input-DMA
    spin-up so real matmuls run at 2.4GHz (216ns per N=512 bf16 MM).
  - Inputs on the Sync HWDGE queue in-order: bands, then strip chunks
    [2,4,6,7,7,8] sized so arrival stays ahead of a warm PE.
  - SWDGE/HWDGE stores have ~10us completion-receipt latency and only 8
    in-flight semaphore lanes, so stores are per-strip (125KB), issued
    right after each strip's drain, round-robined over the GpSimd/Sync/
    Scalar queues so no single queue's receipt pipeline backs up and the
    final receipt follows the last drain closely.
  - PSUM groups [2,2,4,4...,2,2,1,1]; dj outer within a group; drains
    alternate Vector/Scalar per strip (single producer per store).

Sharding: output columns split across the 8 cores (512 cols/core); each
core processes all 4090 output rows. Kernel + bias replicated.
"""

import numpy as np

H, W = 4096, 4096
KH, KW = 7, 7
OH, OW = H - KH + 1, W - KW + 1  # 4090, 4090
N_CORES = 8
CW = 512               # output columns per core
IW = CW + KW - 1       # input columns per core (518)
STRIP = 122            # output rows per strip (K = STRIP + KH - 1 = 128)
MB = 128               # stationary block columns (M padded 122 -> 128)
N_STRIPS = (OH + STRIP - 1) // STRIP  # 34 (last strip M=64, K=70)

GROUPS = [2, 2, 4, 4, 4, 4, 4, 4, 2, 2, 1, 1]   # strips per PSUM group
IN_CHUNKS = [2, 4, 6, 7, 7, 8]                  # strips per input DMA
N_WARM = 10                                      # dummy matmuls for HAM warmup

assert sum(GROUPS) == N_STRIPS and sum(IN_CHUNKS) == N_STRIPS

_cache = {}


def _build_nc():
    import concourse.bacc as bacc
    import concourse.mybir as mybir
    from concourse.tile import TileContext

    f32 = mybir.dt.float32
    bf16 = mybir.dt.bfloat16

    nc = bacc.Bacc("TRN2", target_bir_lowering=False, debug=False)
    xs = nc.dram_tensor("xs", [128, N_STRIPS * IW], bf16, kind="ExternalInput")
    bands = nc.dram_tensor("bands", [128, KW * MB], bf16, kind="ExternalInput")
    biasv = nc.dram_tensor("biasv", [128, 1], f32, kind="ExternalInput")
    # Packed output: out[m, s*CW + c] = out_full[122*s + m, c]; host unpacks.
    out = nc.dram_tensor("out", [STRIP, N_STRIPS * CW], bf16, kind="ExternalOutput")

    with TileContext(nc) as tc:
        with (
            tc.tile_pool(name="const", bufs=1) as cpool,
            tc.tile_pool(name="rhs", bufs=6) as rpool,
            tc.tile_pool(name="obuf", bufs=8) as opool,
            tc.tile_pool(name="psum", bufs=8, space="PSUM") as ppool,
        ):
            # Warmup scratch on GpSimd (earliest-free engine) so the PE's
            # warmup burst starts as soon as possible after the preamble.
            warm_t = cpool.tile([128, 640], bf16)
            nc.gpsimd.memset(warm_t[:, :], 0.0)
            bias1_t = cpool.tile([128, 1], f32)
            nc.scalar.dma_start(out=bias1_t[:, :], in_=biasv[:, :])

            # Input loads on the Sync HWDGE queue, in-order, bands first.
            band_t = cpool.tile([128, KW * MB], bf16)
            nc.sync.dma_start(out=band_t[:, :], in_=bands[:, :])
            strip_tile = {}
            s0 = 0
            for n in IN_CHUNKS:
                xt = rpool.tile([128, max(IN_CHUNKS) * IW], bf16, tag="rhs")
                nc.sync.dma_start(
                    out=xt[:, : n * IW], in_=xs[:, s0 * IW : (s0 + n) * IW]
                )
                for j in range(n):
                    strip_tile[s0 + j] = (xt, j * IW)
                s0 += n

            # HAM warmup: ~4us of dummy matmuls (cold: 427ns each) so the PE
            # clock is at 2.4GHz when real matmuls start.
            warm_ps = ppool.tile([128, CW], f32, name="ps", tag="ps")
            for _ in range(N_WARM):
                nc.tensor.matmul(
                    warm_ps[:, :],
                    warm_t[:, :128],
                    warm_t[:, 128:640],
                    start=True,
                    stop=True,
                )
            # broadcast bias to [128, CW] on-chip for the Vector drains
            bias_t = cpool.tile([128, CW], f32)
            nc.vector.tensor_scalar_add(
                bias_t[:, :], warm_t[:, :CW], bias1_t[:, :1]
            )

            store_q = [nc.gpsimd, nc.sync, nc.scalar]
            strips_done = 0
            for gi, n in enumerate(GROUPS):
                s0 = strips_done
                strips = list(range(s0, s0 + n))
                strips_done += n
                dims = []
                for s in strips:
                    r0 = s * STRIP
                    dims.append((r0, min(STRIP, OH - r0), min(128, H - r0)))
                ps_ts = [
                    ppool.tile([128, CW], f32, name="ps", tag="ps") for _ in strips
                ]
                for dj in range(KW):
                    lhsT = band_t[:, dj * MB : dj * MB + MB]
                    for j, (r0, M, K) in enumerate(dims):
                        sxt, soff = strip_tile[strips[j]]
                        nc.tensor.matmul(
                            ps_ts[j][:, :],
                            lhsT[:K, :],
                            sxt[:K, soff + dj : soff + dj + CW],
                            start=(dj == 0),
                            stop=(dj == KW - 1),
                        )
                # Per-strip drain (alternating engines) + per-strip store
                # (rotating queues): each store has one producer and its own
                # receipt pipeline; ~125KB stores every ~1.5us steady-state.
                for j, (r0, M, K) in enumerate(dims):
                    s = strips[j]
                    ot = opool.tile([128, CW], bf16, tag="ot")
                    if s % 2 == 0:
                        nc.vector.tensor_tensor(
                            ot[:M, :],
                            ps_ts[j][:M, :],
                            bias_t[:M, :],
                            mybir.AluOpType.add,
                        )
                    else:
                        nc.scalar.activation(
                            ot[:M, :],
                            ps_ts[j][:M, :],
                            mybir.ActivationFunctionType.Identity,
                            bias=bias1_t[:M, :1],
                        )
                    store_q[s % 3].dma_start(
                        out=out[:, s * CW : (s + 1) * CW],
                        in_=ot[:STRIP, :],
                    )

    nc.finalize()
    return nc


def _get_nc():
    if "nc" not in _cache:
        _cache["nc"] = _build_nc()
    return _cache["nc"]


def _build_bands(weight: np.ndarray) -> np.ndarray:
    """bands[k, dj*MB + m] = weight[k - m, dj] for 0 <= k-m < KH, m < STRIP."""
    w = np.asarray(weight, np.float32)
    bands = np.zeros((128, KW * MB), np.float32)
    m = np.arange(STRIP)
    for dj in range(KW):
        for di in range(KH):
            bands[m + di, dj * MB + m] = w[di, dj]
    return bands


def _prepare_in_maps(x, weight, bias):
    import ml_dtypes

    bf16 = ml_dtypes.bfloat16
    xb = np.ascontiguousarray(x, np.float32).astype(bf16)
    bands = _build_bands(weight).astype(bf16)
    bias_tile = np.full((128, 1), np.float32(np.asarray(bias).reshape(-1)[0]))

    # xs_packed[k, s, c] = x[122*s + k, c0 + c], zero beyond image edges.
    k_idx = np.arange(128)[:, None]
    s_idx = np.arange(N_STRIPS)[None, :]
    rows = k_idx + STRIP * s_idx  # [128, N_STRIPS]
    row_ok = rows < H
    rows_c = np.minimum(rows, H - 1)

    in_maps = []
    for c in range(N_CORES):
        c0 = c * CW
        avail = min(IW, W - c0)
        xsl = np.zeros((H, IW), bf16)
        xsl[:, :avail] = xb[:, c0 : c0 + avail]
        xs = xsl[rows_c, :]  # [128, N_STRIPS, IW]
        xs[~row_ok] = 0
        xs = np.ascontiguousarray(xs.reshape(128, N_STRIPS * IW))
        in_maps.append({"xs": xs, "bands": bands, "biasv": bias_tile})
    return in_maps


def _gather_out(per_core_outs) -> np.ndarray:
    out = np.empty((OH, OW), np.float32)
    for c in range(N_CORES):
        c0 = c * CW
        take = min(CW, OW - c0)
        po = per_core_outs[c]["out"].astype(np.float32).reshape(STRIP, N_STRIPS, CW)
        full = po.transpose(1, 0, 2).reshape(N_STRIPS * STRIP, CW)
        out[:, c0 : c0 + take] = full[:OH, :take]
    return out


def kernel(x: np.ndarray, weight: np.ndarray, bias: np.ndarray) -> np.ndarray:
    from concourse import bass_utils

    nc = _get_nc()
    in_maps = _prepare_in_maps(x, weight, bias)
    res = bass_utils.run_bass_kernel_spmd(nc, in_maps, list(range(N_CORES)))
    _cache["last_results"] = res
    return _gather_out(res.results)


# revision 7
# speedup vs baseline: 1.1969x; 1.1969x over previous
"""Trainium2 Bass kernel: 7x7 valid cross-correlation + bias on a 4096x4096 f32 image.

Formulation: banded matmul on the TensorEngine.
  out[r, c] = sum_{di,dj} w[di,dj] * x[r+di, c+dj]
For an output row-strip of M=122 rows starting at r0, using K=128 input rows:
  out[r0+m, c] = sum_k A_dj[k, m] * x[r0+k, c+dj]   summed over dj=0..6
where A_dj[k, m] = w[k-m, dj] for 0 <= k-m < 7 (a banded [128, 122] matrix,
precomputed on host from the 49 kernel weights). The 7 dj-terms accumulate
into one PSUM bank via shifted column slices of the same SBUF rhs tile.

Matmuls run in bf16; output written back as bf16, upcast on host
(rel-err ~4e-3 vs the 2e-2 gate).

Schedule (v4, trace-driven):
  - exec_time spans first USER instruction -> end of NEFF epilogue; the
    framework preamble is free, the trailing semaphore sweep is not.
  - HAM warmup: dummy matmuls on a memset scratch during the input-DMA
    spin-up so real matmuls run at 2.4GHz (216ns per N=512 bf16 MM).
  - Inputs on the Sync HWDGE queue in-order: bands, then strip chunks
    [2,4,6,7,7,8] sized so arrival stays ahead of a warm PE.
  - SWDGE/HWDGE stores have ~10us completion-receipt latency and only 8
    in-flight semaphore lanes, so stores are per-strip (125KB), issued
    right after each strip's drain, round-robined over the GpSimd/Sync/
    Scalar queues so no single queue's receipt pipeline backs up and the
    final receipt follows the last drain closely.
  - PSUM groups [2,2,4,4...,2,2,1,1]; dj outer within a group; drains
    alternate Vector/Scalar per strip (single producer per store).

Sharding: output columns split across the 8 cores (512 cols/core); each
core processes all 4090 output rows. Kernel + bias replicated.
"""

import numpy as np

H, W = 4096, 4096
KH, KW = 7, 7
OH, OW = H - KH + 1, W - KW + 1  # 4090, 4090
N_CORES = 8
CW = 512               # output columns per core
IW = CW + KW - 1       # input columns per core (518)
STRIP = 122            # output rows per strip (K = STRIP + KH - 1 = 128)
MB = 128               # stationary block columns (M padded 122 -> 128)
N_STRIPS = (OH + STRIP - 1) // STRIP  # 34 (last strip M=64, K=70)

GROUPS = [2, 2, 4, 4, 4, 4, 4, 4, 2, 2, 1, 1]   # strips per PSUM group
IN_CHUNKS = [2, 4, 6, 7, 7, 8]                  # strips per input DMA
N_WARM = 10                                      # dummy matmuls for HAM warmup

assert sum(GROUPS) == N_STRIPS and sum(IN_CHUNKS) == N_STRIPS

_cache = {}


def _build_nc():
    import concourse.bacc as bacc
    import concourse.mybir as mybir
    from concourse.tile import TileContext

    f32 = mybir.dt.float32
    bf16 = mybir.dt.bfloat16

    nc = bacc.Bacc("TRN2", target_bir_lowering=False, debug=False)
    xs = nc.dram_tensor("xs", [128, N_STRIPS * IW], bf16, kind="ExternalInput")
    bands = nc.dram_tensor("bands", [128, KW * MB], bf16, kind="ExternalInput")
    biasv = nc.dram_tensor("biasv", [128, 1], f32, kind="ExternalInput")
    # Packed output: out[m, s*CW + c] = out_full[122*s + m, c]; host unpacks.
    out = nc.dram_tensor("out", [STRIP, N_STRIPS * CW], bf16, kind="ExternalOutput")

    with TileContext(nc) as tc:
        with (
            tc.tile_pool(name="const", bufs=1) as cpool,
            tc.tile_pool(name="rhs", bufs=6) as rpool,
            tc.tile_pool(name="obuf", bufs=8) as opool,
            tc.tile_pool(name="psum", bufs=8, space="PSUM") as ppool,
        ):
            # Warmup scratch on GpSimd (earliest-free engine) so the PE's
            # warmup burst starts as soon as possible after the preamble.
            warm_t = cpool.tile([128, 640], bf16)
            nc.gpsimd.memset(warm_t[:, :], 0.0)
            bias1_t = cpool.tile([128, 1], f32)
            nc.scalar.dma_start(out=bias1_t[:, :], in_=biasv[:, :])

            # Input loads on the Sync HWDGE queue, in-order, bands first.
            band_t = cpool.tile([128, KW * MB], bf16)
            nc.sync.dma_start(out=band_t[:, :], in_=bands[:, :])
            strip_tile = {}
            s0 = 0
            for n in IN_CHUNKS:
                xt = rpool.tile([128, max(IN_CHUNKS) * IW], bf16, tag="rhs")
                nc.sync.dma_start(
                    out=xt[:, : n * IW], in_=xs[:, s0 * IW : (s0 + n) * IW]
                )
                for j in range(n):
                    strip_tile[s0 + j] = (xt, j * IW)
                s0 += n

            # HAM warmup: ~4us of dummy matmuls (cold: 427ns each) so the PE
            # clock is at 2.4GHz when real matmuls start.
            warm_ps = ppool.tile([128, CW], f32, name="ps", tag="ps")
            for _ in range(N_WARM):
                nc.tensor.matmul(
                    warm_ps[:, :],
                    warm_t[:, :128],
                    warm_t[:, 128:640],
                    start=True,
                    stop=True,
                )
            # broadcast bias to [128, CW] on-chip for the Vector drains
            bias_t = cpool.tile([128, CW], f32)
            nc.vector.tensor_scalar_add(
                bias_t[:, :], warm_t[:, :CW], bias1_t[:, :1]
            )

            strips_done = 0
            for gi, n in enumerate(GROUPS):
                s0 = strips_done
                strips = list(range(s0, s0 + n))
                strips_done += n
                dims = []
                for s in strips:
                    r0 = s * STRIP
                    dims.append((r0, min(STRIP, OH - r0), min(128, H - r0)))
                ps_ts = [
                    ppool.tile([128, CW], f32, name="ps", tag="ps") for _ in strips
                ]
                for dj in range(KW):
                    lhsT = band_t[:, dj * MB : dj * MB + MB]
                    for j, (r0, M, K) in enumerate(dims):
                        sxt, soff = strip_tile[strips[j]]
                        nc.tensor.matmul(
                            ps_ts[j][:, :],
                            lhsT[:K, :],
                            sxt[:K, soff + dj : soff + dj + CW],
                            start=(dj == 0),
                            stop=(dj == KW - 1),
                        )
                # Per-strip drain (alternating engines) + per-strip store
                # (rotating queues): each store has one producer and its own
                # receipt pipeline; ~125KB stores every ~1.5us steady-state.
                for j, (r0, M, K) in enumerate(dims):
                    s = strips[j]
                    ot = opool.tile([128, CW], bf16, tag="ot")
                    if s % 2 == 0:
                        nc.vector.tensor_tensor(
                            ot[:M, :],
                            ps_ts[j][:M, :],
                            bias_t[:M, :],
                            mybir.AluOpType.add,
                        )
                    else:
                        nc.scalar.activation(
                            ot[:M, :],
                            ps_ts[j][:M, :],
                            mybir.ActivationFunctionType.Identity,
                            bias=bias1_t[:M, :1],
                        )
                    # SWDGE only: HWDGE SBUF->HBM stores complete far too
                    # slowly on this platform (v4 measured receipts at +25us),
                    # and stores on Sync/Scalar also delay input chunks and
                    # drains sharing those queues. Small per-strip stores keep
                    # receipts short and lanes recycling.
                    nc.gpsimd.dma_start(
                        out=out[:, s * CW : (s + 1) * CW],
                        in_=ot[:STRIP, :],
                    )

    nc.finalize()
    return nc


def _get_nc():
    if "nc" not in _cache:
        _cache["nc"] = _build_nc()
    return _cache["nc"]


def _build_bands(weight: np.ndarray) -> np.ndarray:
    """bands[k, dj*MB + m] = weight[k - m, dj] for 0 <= k-m < KH, m < STRIP."""
    w = np.asarray(weight, np.float32)
    bands = np.zeros((128, KW * MB), np.float32)
    m = np.arange(STRIP)
    for dj in range(KW):
        for di in range(KH):
            bands[m + di, dj * MB + m] = w[di, dj]
    return bands


def _prepare_in_maps(x, weight, bias):
    import ml_dtypes

    bf16 = ml_dtypes.bfloat16
    xb = np.ascontiguousarray(x, np.float32).astype(bf16)
    bands = _build_bands(weight).astype(bf16)
    bias_tile = np.full((128, 1), np.float32(np.asarray(bias).reshape(-1)[0]))

    # xs_packed[k, s, c] = x[122*s + k, c0 + c], zero beyond image edges.
    k_idx = np.arange(128)[:, None]
    s_idx = np.arange(N_STRIPS)[None, :]
    rows = k_idx + STRIP * s_idx  # [128, N_STRIPS]
    row_ok = rows < H
    rows_c = np.minimum(rows, H - 1)

    in_maps = []
    for c in range(N_CORES):
        c0 = c * CW
        avail = min(IW, W - c0)
        xsl = np.zeros((H, IW), bf16)
        xsl[:, :avail] = xb[:, c0 : c0 + avail]
        xs = xsl[rows_c, :]  # [128, N_STRIPS, IW]
        xs[~row_ok] = 0
        xs = np.ascontiguousarray(xs.reshape(128, N_STRIPS * IW))
        in_maps.append({"xs": xs, "bands": bands, "biasv": bias_tile})
    return in_maps


def _gather_out(per_core_outs) -> np.ndarray:
    out = np.empty((OH, OW), np.float32)
    for c in range(N_CORES):
        c0 = c * CW
        take = min(CW, OW - c0)
        po = per_core_outs[c]["out"].astype(np.float32).reshape(STRIP, N_STRIPS, CW)
        full = po.transpose(1, 0, 2).reshape(N_STRIPS * STRIP, CW)
        out[:, c0 : c0 + take] = full[:OH, :take]
    return out


def kernel(x: np.ndarray, weight: np.ndarray, bias: np.ndarray) -> np.ndarray:
    from concourse import bass_utils

    nc = _get_nc()
    in_maps = _prepare_in_maps(x, weight, bias)
    res = bass_utils.run_bass_kernel_spmd(nc, in_maps, list(range(N_CORES)))
    _cache["last_results"] = res
    return _gather_out(res.results)
